# revision 1
# baseline (speedup 1.0000x reference)
"""Trainium2 Bass kernel for nn_BlockCrossAttn (block-diagonal attention, E=H=1).

Math per (block b, batch n) pair (256-long vectors q', k', v of the block):
    q' = wq*Q + bq ; k' = wk*K + bk
    soft[q,k] = softmax_k(q'[q] * k'[k])
    out[q] = wvo * (sum_k soft[q,k] * V[k]) + (bvo + bo)
where wvo = wo*wv, bvo = wo*bv (the V/out affine folds into the epilogue
because softmax weights sum to 1).  No max-subtraction: |scores| <= ~27
worst case, exp is safe in fp32.

Sharding: 128 blocks of 256 rows; 16 blocks per core across 8 cores
(fully independent, no collectives).

Per-core device pipeline (512 pairs):
  - PE outer products (contraction dim 1) build S^T[k, q] in PSUM,
    3 pairs per 3-bank group, double buffered.
  - ScalarE exp over [128, 1536] PSUM spans -> E in SBUF.
  - PE reduction matmuls: lhsT = [ones, v] 2-column AP (arbitrary free
    stride), rhs = E streams -> PSUM [2, 256] = (denom, numer) rows per
    pair; every matmul start=True/stop=True (no PSUM accumulation -> no
    whole-bank has_written hazards); 4 pairs per result bank.
  - VectorE flushes banks to SBUF; a DRAM scratch bounce re-lays 32 pairs
    into a dense [32, 1024] tile (one writer); VectorE adds the two ktile
    partials, reciprocal_approx_fast + multiply + affine epilogue;
    one contiguous DMA per block to the n-major output.

Weight scalars are baked into the module as immediates (compiled per
weight set, cached) to avoid TensorScalarPtr sync-wait limits.
"""

from contextlib import ExitStack

import numpy as np

import concourse.bacc as bacc
import concourse.bass as bass
import concourse.tile as tile
from concourse import mybir
from concourse.bass_utils import run_bass_kernel_spmd

FP = mybir.dt.float32
AF = mybir.ActivationFunctionType
ALU = mybir.AluOpType

L = 32768          # sequence length
N = 32             # batch
BS = 256           # block size
NB = L // BS       # 128 blocks
NCORES = 8
BPC = NB // NCORES  # 16 blocks per core
LS = BPC * BS       # 4096 rows per core shard

GROUP = 3           # pairs per exp staging group (3 PSUM banks)
PAIRS = BPC * N     # 512 pairs per core
F32R = mybir.dt.float32r
BF16 = mybir.dt.bfloat16
F32R_OUTER = True   # full-rate relaxed-precision fp32 matmul for scores
EDT = BF16          # E dtype for the reductions (rounding cancels in ratio)


def build_kernel_module(sc, reps: int = 1) -> bass.Bass:
    """sc: dict of python-float weight scalars baked as immediates.

    reps > 1 wraps the whole body in a device-side For_i loop — used only
    for wall-clock benchmarking (amplifies device time over dispatch noise).
    """
    nc = bacc.Bacc("TRN2", target_bir_lowering=False, debug=False, num_devices=NCORES)
    # qkt[4n+c, :] = [qT[n, 1024c:1024(c+1)] | kT[n, 1024c:1024(c+1)]]
    qkt = nc.declare_dram_parameter("qkt", [128, 2048], FP, isOutput=False)
    v = nc.declare_dram_parameter("v", [LS, N], FP, isOutput=False)
    kraw = nc.declare_dram_parameter("kraw", [LS, N], FP, isOutput=False)
    out_t = nc.declare_dram_parameter("out_t", [N, LS], FP, isOutput=True)

    with tile.TileContext(nc) as tc:
        with ExitStack() as ctx:
            if reps == 1:
                _emit(ctx, tc, qkt, v, kraw, out_t, sc)
            else:
                with tc.For_i(0, reps, 1):
                    _emit(ctx, tc, qkt, v, kraw, out_t, sc)
    nc.compile()
    return nc


def _emit(ctx, tc, qkt, v, kraw, out_t, sc):
    nc = tc.nc

    rows = ctx.enter_context(tc.tile_pool(name="rows", bufs=1))
    stage = ctx.enter_context(tc.tile_pool(name="stage", bufs=2))
    vpool = ctx.enter_context(tc.tile_pool(name="vpool", bufs=1))
    epool = ctx.enter_context(tc.tile_pool(name="epool", bufs=3))
    dpool = ctx.enter_context(tc.tile_pool(name="dpool", bufs=2))
    spool = ctx.enter_context(tc.tile_pool(name="spool", bufs=2))
    qpool = ctx.enter_context(tc.tile_pool(name="qpool", bufs=3))
    ps_stage = ctx.enter_context(tc.tile_pool(name="ps_stage", bufs=2, space="PSUM"))
    ps_res = ctx.enter_context(tc.tile_pool(name="ps_res", bufs=2, space="PSUM"))
    drs = ctx.enter_context(tc.tile_pool(name="drs", bufs=2, space="DRAM"))

    # --- prep ------------------------------------------------------------------
    QKDT = F32R if F32R_OUTER else FP
    qk4 = rows.tile([128, 2048], QKDT, name="qk4", tag="qk4")
    nc.sync.dma_start(out=qk4[:].bitcast(FP), in_=qkt[:])
    nc.vector.tensor_scalar(
        out=qk4[:, 0:1024], in0=qk4[:, 0:1024].bitcast(FP),
        scalar1=sc["wq"], scalar2=sc["bq"], op0=ALU.mult, op1=ALU.add,
    )
    nc.vector.tensor_scalar(
        out=qk4[:, 1024:2048], in0=qk4[:, 1024:2048].bitcast(FP),
        scalar1=sc["wk"], scalar2=sc["bk"], op0=ALU.mult, op1=ALU.add,
    )

    # k' transposed onto partitions for the GpSimd/DVE score path:
    # kcol[p, b*64 + t*32 + n] = k'[b*256 + t*128 + p, n].  Loaded from the
    # raw [LS, N] k shard so both DMA sides have matched 128B n-runs (the
    # qkt-side transpose degenerates to 4-byte descriptors, ~5us per DMA).
    kcol = rows.tile([128, BPC * 2 * N], FP, name="kcol", tag="kcol")
    kin = kraw[:].rearrange("(b t p) n -> b t p n", t=2, p=128)
    kout = kcol[:].rearrange("p (b t n) -> b t p n", b=BPC, t=2)

    def emit_kcol_block(kb):
        # One block's kcol slice: 2 small DMAs (~1.2us of Sync queue) plus a
        # per-block affine.  Interleaved after the early row-staging DMAs so
        # the Sync queue's just-in-time staging is never delayed, and each
        # block's slice is ready well before its offloaded groups need it.
        for t in (0, 1):  # partition dim must lead the DMA out AP
            nc.sync.dma_start(out=kout[kb, t], in_=kin[kb, t])
        ksl = kcol[:][:, kb * 64:(kb + 1) * 64]
        nc.vector.tensor_scalar(
            out=ksl, in0=ksl,
            scalar1=sc["wk"], scalar2=sc["bk"], op0=ALU.mult, op1=ALU.add,
        )

    # [ones, v] tiles: col 0 = 1.0 (memset once); cols 1..64 = raw V of the
    # block, [t, n] order.  Two fixed tiles used alternately per block.
    vcombs = []
    for name in ("vcA", "vcB"):
        vc = vpool.tile([128, 2, N, 3], EDT, name=name, tag=name)
        nc.vector.memset(vc[:], 1.0)
        vcombs.append(vc)

    def load_vcomb(b):
        # DMA raw V, then split into bf16 hi+lo columns (exact to ~2^-16).
        vc = vcombs[b % 2]
        vch = vpool.tile([128, 2, N], FP, name="vch", tag="vch", bufs=2)
        hi32 = vpool.tile([128, 2, N], FP, name="hi32", tag="hi32", bufs=2)
        nc.sync.dma_start(
            out=vch[:],
            in_=v[b * BS:(b + 1) * BS, :].rearrange("(t p) n -> p t n", p=128),
        )
        vc4 = vc[:]
        nc.vector.tensor_copy(vc4[:, :, :, 1], vch[:])
        nc.vector.tensor_copy(hi32[:], vc4[:, :, :, 1])
        nc.vector.tensor_sub(vc4[:, :, :, 2], vch[:], hi32[:])
        return vc

    # --- per-half-block q/k row staging (to partition 0) -----------------------
    def stage_rows(b, h):
        # row n (16h <= n < 16h+16): q at [0, (2(n-16h))*256:...],
        #                            k at [0, (2(n-16h)+1)*256:...]
        qks = stage.tile([1, 16 * 2 * BS], QKDT, name="qks", tag="qks")
        qv = qk4[:].rearrange("(n c) (g f) -> n c g f", c=4, g=2)
        cb, cc = b // 4, (b % 4) * BS
        nc.sync.dma_start(out=qks[:], in_=qv[16 * h:16 * (h + 1), cb, :, cc:cc + BS])
        return qks

    # --- main loop --------------------------------------------------------------
    vcur = [None]
    res_state = {"tile": None, "count": 0, "nflush": 0, "rs": None, "first_g": 0}

    def emit_reduces(pend):
        e, members = pend
        for (s, b, n, vc) in members:
            g = b * N + n
            r = res_state["count"]
            if r == 0:
                res_state["tile"] = ps_res.tile([128, 512], FP, name="res", tag="res")
                if res_state["nflush"] == 0:
                    res_state["rs"] = dpool.tile([128, 4096], FP, name="rs", tag="rs")
                    res_state["first_g"] = g
            jj = r
            for t in (0, 1):
                nc.tensor.matmul(
                    res_state["tile"][32 * jj:32 * jj + 3, t * 256:(t + 1) * 256],
                    lhsT=vc[:][:, t, n, :],
                    rhs=e[:][:, s * 512 + t * 256: s * 512 + (t + 1) * 256],
                    start=True, stop=True,
                    tile_position=(0, 32 * jj),
                )
            res_state["count"] += 1
            if res_state["count"] == 4:
                m = res_state["nflush"]
                nc.vector.tensor_copy(
                    res_state["rs"][:, m * 512:(m + 1) * 512], res_state["tile"][:]
                )
                res_state["count"] = 0
                res_state["tile"] = None
                res_state["nflush"] += 1
                if res_state["nflush"] == 8:
                    division_batch()

    def division_batch():
        b0 = res_state["first_g"] // N
        rs = res_state["rs"]
        # rows {32j+r} of rs -> DRAM scratch already in dense layout:
        # scr[4m+j, r*512 + tq] ; then scratch -> dn is a contiguous copy.
        scr = drs.tile([N, 1536], FP, name="scr", tag="scr")
        rsv = rs[:].rearrange("(j p2) (m tq) -> j p2 m tq", j=4, m=8)
        sw = scr[:].rearrange("(m j) (r tq) -> j m r tq", m=8, r=3)
        for r in (0, 1, 2):
            nc.sync.dma_start(out=sw[:, :, r, :], in_=rsv[:, r, :, :])
        # scratch -> dense [32, 1536]: partition 4m+j (= local pair n), free (r,t,q)
        dn = dpool.tile([N, 1536], FP, name="dn", tag="dn")
        nc.sync.dma_start(out=dn[:], in_=scr[:])
        dnv = dn[:].rearrange("p (r t q) -> p r t q", r=3, t=2)
        den = dpool.tile([N, BS], FP, name="den", tag="den")
        num = dpool.tile([N, BS], FP, name="num", tag="num")
        nc.vector.tensor_add(den[:], dnv[:, 0, 0, :], dnv[:, 0, 1, :])
        nc.vector.tensor_add(num[:], dnv[:, 1, 0, :], dnv[:, 1, 1, :])
        nc.vector.tensor_add(num[:], num[:], dnv[:, 2, 0, :])
        nc.vector.tensor_add(num[:], num[:], dnv[:, 2, 1, :])
        nc.vector.reciprocal_approx_fast(out=den[:], in_=den[:])
        ov = dpool.tile([N, BS], FP, name="ov", tag="ov")
        nc.vector.tensor_mul(ov[:], num[:], den[:])
        nc.vector.tensor_scalar(
            out=ov[:], in0=ov[:], scalar1=sc["wvo"], scalar2=sc["bvo"] + sc["bo"],
            op0=ALU.mult, op1=ALU.add,
        )
        nc.sync.dma_start(out=out_t[:, b0 * BS:(b0 + 1) * BS], in_=ov[:])
        res_state["nflush"] = 0
        res_state["rs"] = None

    pending = None
    cur_stage = None
    cur_rows = None
    members = []
    # Offloaded-group placement: every 4th group, but shifted +2 when it
    # falls on a block boundary — the division-chain hiccup there already
    # starves the PE, so removing its outer products too widens the stall.
    boundary = {-(-32 * bb // 3) for bb in range(BPC)}
    off_set = set()
    for ogi in range(2, PAIRS // GROUP, 4):
        shifted = ogi + 2 if any(abs(ogi - bb) <= 1 for bb in boundary) else ogi
        if shifted * GROUP + GROUP <= PAIRS:
            off_set.add(shifted)
    for g in range(PAIRS):
        b, n = divmod(g, N)
        if n == 0:
            vcur[0] = load_vcomb(b)
        if n % 16 == 0:
            cur_rows = stage_rows(b, n // 16)
            if g // 16 < BPC:
                emit_kcol_block(g // 16)
        qks = cur_rows
        nn = n % 16
        s = g % GROUP
        gi = g // GROUP
        # Every 4th full group is scored off the PE: GpSimd broadcasts the
        # staged q row across partitions, VectorE multiplies by per-partition
        # k' into an SBUF tile.  The exp and reduction paths are identical.
        # kcol block kb loads at chunk kb, at or before its first use (chunk
        # 2*kb); gi >= 6 skips only the first eligible group, whose block-0
        # kcol slice would land just-in-time.
        off = gi in off_set
        if s == 0:
            members = []
            if off:
                cur_stage = spool.tile([128, GROUP * 512], FP, name="sg", tag="sg")
            else:
                cur_stage = ps_stage.tile([128, GROUP * 512], FP, name="st", tag="st")
        if off:
            qb = qpool.tile([128, 256], FP, name="qb", tag="qb")
            # partition_broadcast's in_ap must be a single-partition row (the
            # ucode walks numel(src) write columns) — read the staged q row.
            qrow = qks[:].bitcast(FP)[0:1, (2 * nn) * BS:(2 * nn + 1) * BS]
            nc.gpsimd.partition_broadcast(qb[:], qrow)
            for t in (0, 1):
                kc = kcol[:][:, b * 64 + t * 32 + n: b * 64 + t * 32 + n + 1]
                nc.vector.tensor_tensor(
                    out=cur_stage[:, s * 512 + t * 256: s * 512 + (t + 1) * 256],
                    in0=qb[:], in1=kc.broadcast_to([128, 256]), op=ALU.mult,
                )
        else:
            for t in (0, 1):
                lhsT = qks[:][0:1, (2 * nn + 1) * BS + t * 128: (2 * nn + 1) * BS + (t + 1) * 128]
                rhs = qks[:][0:1, (2 * nn) * BS: (2 * nn + 1) * BS]
                nc.tensor.matmul(
                    cur_stage[:, s * 512 + t * 256: s * 512 + (t + 1) * 256],
                    lhsT=lhsT, rhs=rhs,
                    start=True, stop=True,
                    tile_position=(0, 0),
                )
        members.append((s, b, n, vcur[0]))
        if s == GROUP - 1 or g == PAIRS - 1:
            e = epool.tile([128, GROUP * 512], EDT, name="e", tag="e")
            width = len(members) * 512
            nc.scalar.activation(e[:][:, 0:width], cur_stage[:][:, 0:width], AF.Exp)
            if pending is not None:
                emit_reduces(pending)
            pending = (e, members)
    emit_reduces(pending)
    assert res_state["count"] == 0 and res_state["nflush"] == 0, (
        "pair count must be a multiple of 32 (one block per division batch)"
    )


_CACHE: dict = {}


def _get_nc(sc, reps: int = 1) -> bass.Bass:
    key = (tuple(sorted(sc.items())), reps)
    if key not in _CACHE:
        _CACHE[key] = build_kernel_module(sc, reps)
    return _CACHE[key]


def make_in_maps(query, key, value, in_proj_w, in_proj_b, out_proj_w, out_proj_b):
    q = np.ascontiguousarray(np.asarray(query, dtype=np.float32).reshape(L, N))
    k = np.ascontiguousarray(np.asarray(key, dtype=np.float32).reshape(L, N))
    vv = np.ascontiguousarray(np.asarray(value, dtype=np.float32).reshape(L, N))
    wq, wk, wv = [float(x) for x in np.asarray(in_proj_w, dtype=np.float32).reshape(3)]
    bq, bk, bv = [float(x) for x in np.asarray(in_proj_b, dtype=np.float32).reshape(3)]
    wo = float(np.asarray(out_proj_w, dtype=np.float32).reshape(1)[0])
    bo = float(np.asarray(out_proj_b, dtype=np.float32).reshape(1)[0])
    sc = {"wq": wq, "bq": bq, "wk": wk, "bk": bk,
          "wvo": float(np.float32(wo) * np.float32(wv)),
          "bvo": float(np.float32(wo) * np.float32(bv)), "bo": bo}
    in_maps = []
    for c in range(NCORES):
        sl = slice(c * LS, (c + 1) * LS)
        qr = np.ascontiguousarray(q[sl].T).reshape(N, 4, LS // 4)
        kr = np.ascontiguousarray(k[sl].T).reshape(N, 4, LS // 4)
        qkt_np = np.concatenate([qr, kr], axis=2).reshape(128, 2048)
        in_maps.append({
            "qkt": np.ascontiguousarray(qkt_np),
            "v": np.ascontiguousarray(vv[sl]),
            "kraw": np.ascontiguousarray(k[sl]),
        })
    return in_maps, sc


def run(in_maps, sc, **kwargs):
    return run_bass_kernel_spmd(_get_nc(sc), in_maps, list(range(NCORES)), **kwargs)


def assemble(results) -> np.ndarray:
    outs = [np.asarray(results[c]["out_t"], dtype=np.float32).T for c in range(NCORES)]
    return np.ascontiguousarray(np.concatenate(outs, axis=0)).reshape(L, N, 1)


def kernel(query, key, value, in_proj_w, in_proj_b, out_proj_w, out_proj_b):
    in_maps, sc = make_in_maps(
        query, key, value, in_proj_w, in_proj_b, out_proj_w, out_proj_b
    )
    res = run(in_maps, sc)
    return assemble(res.results)

